# revision 7
# baseline (speedup 1.0000x reference)
"""Binarized conv1d (k=7, pad=3 with -1.0) + maxpool(2) + PReLU + BatchNorm1d
(training stats) fused Trainium2 kernel, data-parallel over batch N across 8
NeuronCores with an on-chip AllReduce for the BN batch statistics.

Contract: kernel(**inputs) takes the FULL inputs from setup_inputs() and
returns the FULL [128, 128, 2048] float32 output.

Algorithm per core (16 of the 128 batches), v3 (fp8 DoubleRow rewrite):
  - activations encoded h' = (x>=0) - 0.5 in {+-0.5} fp8e4 (pad -0.5), so the
    +-1 binarized conv is exactly 2*conv(h', sign(W)); the 2x folds into the
    BN statistics scalar math. h' is one vector-engine tensor_scalar per
    batch (f32 single-src runs in the 2-port DVE mode, ~2 elem/cycle).
    GPSIMD is kept off the hot path entirely: concurrent GPSIMD instructions
    were measured to block DVE accum_out ops for their full duration.
  - the conv+maxpool is computed as two stride-2 convs, E (even positions)
    and O (odd positions, tap weights shifted by one); since prelu is
    monotone, maxpool+PReLU = max(prelu(E), prelu(O)): the scalar engine
    applies Prelu to each conv straight out of PSUM (per 1024-wide half, so
    banks free early), and one vector STT takes the max, with accum_out
    emitting the per-channel sum(y') for free.
  - conv matmuls use fp8e4 DoubleRow perf mode: contraction 256 = 64 ch x 4
    taps per matmul (partition (c,g) holds h' shifted by 2g; the in-pair tap
    comes from the DoubleRow pair dim with stride 1, output stride 2). Per
    512-wide psum bank: 2 accumulated matmuls instead of 4 bf16 ones.
    Even/odd batches swap the shifted/direct partition halves so the h'
    write stays lane-aligned; both weight layouts precomputed on the host.
    O-convs are issued first so the scalar engine starts draining PSUM while
    the E-convs still run.
  - sum(y'^2) via one vector STT per batch.
  - BN stats (sum, sumsq) all-reduced across the 8 cores with direct
    remote-DMA SBUF broadcasts (XOR-slot exchange, ~5us); each core computes
    scale/shift (Newton-corrected sqrt) and streams s*y+t back to HBM.
"""

import uuid

import numpy as np
import ml_dtypes
import jax

# The jax persistent compilation cache mis-keys bass_exec custom-call
# executables (the embedded NEFF differs while the cache key does not),
# which can hand back a stale executable and wedge the device. Disable it.
jax.config.update("jax_enable_compilation_cache", False)

import concourse.bacc as bacc
import concourse.mybir as mybir
import concourse.tile as tile
from concourse.bass_utils import run_bass_kernel_spmd

AF = mybir.ActivationFunctionType
ALU = mybir.AluOpType

N_CORES = 8
N = 128            # total batch
NB = N // N_CORES  # batches per core = 16
CI = 64            # in channels
CO = 128           # out channels
L = 4096           # input length
LO = L // 2        # pooled output length = 2048
K = 7              # kernel taps
TW = L + 4         # h' tile width: pads 0:3 / L+3, data 3:L+3
PAD_VAL = -1.0
EPS = 1e-5
M_GLOBAL = float(N * LO)  # BN reduction count per channel

FP8 = mybir.dt.float8e4
FP8NP = mybir.dt.np(FP8)

T_BUFS = 4   # must be even: pad layout alternates with batch parity
XT_BUFS = 3


def _build(alpha: float):
    nc = bacc.Bacc("TRN2", target_bir_lowering=False, debug=False,
                   num_devices=N_CORES)

    xs = nc.dram_tensor("xs", [NB * CI, L], mybir.dt.float32, kind="ExternalInput")
    wts = nc.dram_tensor("wts", [128, 8 * 256], FP8, kind="ExternalInput")
    gb = nc.dram_tensor("gb", [128, 2], mybir.dt.float32, kind="ExternalInput")
    out = nc.dram_tensor("out", [NB * CO, LO], mybir.dt.float32, kind="ExternalOutput")

    with tile.TileContext(nc) as tc:
        with (
            tc.tile_pool(name="wp", bufs=1) as wp,
            tc.tile_pool(name="xp", bufs=XT_BUFS) as xp,
            tc.tile_pool(name="tp", bufs=T_BUFS) as tp,
            tc.tile_pool(name="pp", bufs=1, space="PSUM") as pp,
            tc.tile_pool(name="ep", bufs=3) as ep,
            tc.tile_pool(name="fp", bufs=4) as fp,
            tc.tile_pool(name="yp", bufs=NB) as yp,
            tc.tile_pool(name="sp", bufs=1) as sp,
            tc.tile_pool(name="op", bufs=2) as op_pool,
        ):
            # weights/params on the scalar HWDGE queue so the first x load
            # owns the sync queue from t=0
            wt = wp.tile([128, 8 * 256], FP8)
            nc.scalar.dma_start(wt[:], wts[:])
            gbt = wp.tile([128, 2], mybir.dt.float32)
            nc.scalar.dma_start(gbt[:], gb[:])

            # trigger the Prelu ACT table load during the DMA ramp (the set
            # also serves Relu)
            warm = wp.tile([128, 1], mybir.dt.float32)
            nc.vector.memset(warm[:], 1.0)
            nc.scalar.activation(warm[:], warm[:], AF.Prelu, alpha=alpha)

            _build_pass(nc, tc, xs, out, wt, gbt, alpha,
                        xp, tp, pp, ep, fp, yp, sp, op_pool)

    nc.compile()
    nc.m.name = f"bk{uuid.uuid4().hex[:10]}"
    return nc


def _conv_mms(nc, wt, T, psum_halves, conv_i, parity):
    """Issue the 8 DoubleRow matmuls of one conv (O or E) for one batch:
    j in {0,1} accumulated, halves h in {0,1}, banks bq in {0,1}."""
    for j in range(2):
        off = ((parity * 2 + conv_i) * 2 + j) * 256
        lhsT = wt[:, off:off + 256].rearrange("p (two m) -> p two m", two=2)
        for h in range(2):
            ps = psum_halves[h]
            for bq in range(2):
                base = 4 * j + 2048 * h + 1024 * bq
                rhs = T[:, base:base + 1024].rearrange(
                    "p (n two) -> p two n", two=2)
                nc.tensor.matmul(
                    ps[:, bq * 512:(bq + 1) * 512], lhsT, rhs,
                    start=(j == 0), stop=(j == 1),
                    perf_mode=mybir.MatmulPerfMode.DoubleRow)


def _build_pass(nc, tc, xs, out, wt, gbt, alpha,
                xp, tp, pp, ep, fp, yp, sp, op_pool):
    # stats: cols 0:16 per-batch sum(y') (STT max accum), 16:32 sum(y'^2)
    stats = sp.tile([128, 32], mybir.dt.float32, name="stats", tag="stats")

    y_tiles = []

    def produce_pair(bp):
        """Load x for batch pair bp and build both h' tiles (fp8, shifted)."""
        xt = xp.tile([128, L], mybir.dt.float32, name=f"xt{bp}", tag="xt")
        nc.sync.dma_start(xt[:], xs[bp * 128:(bp + 1) * 128, :])
        Ts = []
        for sub in range(2):
            b = 2 * bp + sub
            # direct half (lane-aligned with the x load), shifted half
            lo, hi = (0, 64) if sub == 0 else (64, 128)
            ol, oh = (64, 128) if sub == 0 else (0, 64)
            T = tp.tile([128, TW], FP8, name=f"T{b}", tag="T")
            if b < T_BUFS:
                # pads persist across buffer reuse (T_BUFS even keeps the
                # parity layout stable per buffer)
                nc.vector.memset(T[lo:hi, 0:3], -0.5)
                nc.vector.memset(T[lo:hi, L + 3:TW], -0.5)
                nc.vector.memset(T[ol:oh, L + 1:TW], -0.5)
            # h' = (x >= 0) - 0.5 in fp8 (DVE single-src 2-port mode)
            nc.vector.tensor_scalar(T[lo:hi, 3:L + 3], xt[lo:hi, :], 0.0, 0.5,
                                    op0=ALU.is_ge, op1=ALU.subtract)
            # shifted-by-2 copy into the other partition half; scalar queue
            # (the sync queue's 2MB x loads would head-of-line block it)
            nc.scalar.dma_start(T[ol:oh, 0:L + 1], T[lo:hi, 2:L + 3])
            Ts.append(T)
        return Ts

    def conv_pair(bp, Ts):
        for sub in range(2):
            b = 2 * bp + sub
            T = Ts[sub]
            ohs = [pp.tile([128, 1024], mybir.dt.float32, name=f"o{b}_{h}",
                           tag=f"o{h}") for h in range(2)]
            ehs = [pp.tile([128, 1024], mybir.dt.float32, name=f"e{b}_{h}",
                           tag=f"e{h}") for h in range(2)]

            # O first: its prelu frees the banks while E still computes
            _conv_mms(nc, wt, T, ohs, 0, sub)
            _conv_mms(nc, wt, T, ehs, 1, sub)

            po = fp.tile([128, LO], mybir.dt.float16, name=f"po{b}", tag="po")
            pe = ep.tile([128, LO], mybir.dt.float16, name=f"pe{b}", tag="pe")
            for h in range(2):
                nc.scalar.activation(po[:, h * 1024:(h + 1) * 1024], ohs[h][:],
                                     AF.Prelu, alpha=alpha)
            for h in range(2):
                nc.scalar.activation(pe[:, h * 1024:(h + 1) * 1024], ehs[h][:],
                                     AF.Prelu, alpha=alpha)

            yt = yp.tile([128, LO], mybir.dt.float16, name=f"yt{b}", tag="yt")
            y_tiles.append(yt)
            nc.vector.scalar_tensor_tensor(
                yt[:], pe[:], 0.0, po[:], op0=ALU.bypass, op1=ALU.max,
                accum_out=stats[:, b:b + 1])
            sq = fp.tile([128, LO], mybir.dt.bfloat16, name=f"sq{b}", tag="po")
            if b % 2 == 0:
                nc.vector.scalar_tensor_tensor(
                    sq[:], yt[:], 1.0, yt[:],
                    op0=ALU.mult, op1=ALU.mult,
                    accum_out=stats[:, 16 + b:17 + b],
                )
            else:
                # ACT takes half the squares to balance the vector engine
                nc.scalar.activation(sq[:], yt[:], AF.Square,
                                     accum_out=stats[:, 16 + b:17 + b])

    Ts_ahead = produce_pair(0)
    for bp in range(NB // 2):
        Ts_cur = Ts_ahead
        if bp + 1 < NB // 2:
            Ts_ahead = produce_pair(bp + 1)
        conv_pair(bp, Ts_cur)

    # ---- local partial stats -> remote-DMA all-reduce -> scale/shift ----
    # Each core broadcasts its [128,2] partial (sum, sumsq) into peer SBUFs:
    # for delta in 1..7, core c sends to core c^delta, landing in xbuf slot
    # delta; the XOR pairing makes every slot single-writer. ~5us instead of
    # the ~55us ncfw AllReduce latency for a 1KB payload.
    loc = sp.tile([128, 2], mybir.dt.float32, name="loc", tag="loc")
    nc.vector.tensor_reduce(loc[:, 0:1], stats[:, 0:16],
                            axis=mybir.AxisListType.X, op=ALU.add)
    nc.vector.tensor_reduce(loc[:, 1:2], stats[:, 16:32],
                            axis=mybir.AxisListType.X, op=ALU.add)

    xbuf = sp.tile([128, 16], mybir.dt.float32, name="xbuf", tag="xbuf")
    nc.vector.tensor_copy(xbuf[:, 0:2], loc[:])
    g = sp.tile([128, 2], mybir.dt.float32, name="g", tag="g")

    rsem = nc.alloc_semaphore("ar_remote")
    psem = nc.alloc_semaphore("ar_prep")
    lsem = nc.alloc_semaphore("ar_local")
    # no_gpsimd_drain: skip the ~45us SWDGE dge_drain at block exit; the
    # kernel-tail drain picks the ring up later, off the critical path.
    with tc.tile_critical(no_gpsimd_drain=True):
        nc.gpsimd.bir_kernel_barrier_wait([list(range(N_CORES))])
        for delta in range(1, 8):
            rd = [None] * 8
            rd[delta] = (0, delta)
            nc.gpsimd.remote_dma_broadcast(
                xbuf[:, 2 * delta:2 * delta + 2], loc[:, 0:2],
                rsem, lsem, rdests=rd,
            ).then_inc(psem, 1)
        nc.gpsimd.wait_ge(psem, 7)
        nc.gpsimd.trigger_dma(count=None)
        # 7 arriving broadcasts x (16//8)=2 incs each
        nc.vector.wait_ge(rsem, 14)
        nc.vector.tensor_reduce(
            g[:, 0:1], xbuf.rearrange("p (s two) -> p two s", two=2)[:, 0:1, :],
            axis=mybir.AxisListType.X, op=ALU.add)
        nc.vector.tensor_reduce(
            g[:, 1:2], xbuf.rearrange("p (s two) -> p two s", two=2)[:, 1:2, :],
            axis=mybir.AxisListType.X, op=ALU.add)

    # mean/var/scale/shift, all [128,1] f32. y = 2*y' so:
    #   mean = 2*S1/M ; E[y^2] = 4*S2/M ; out = (2*s)*y' + (beta - s*mean)
    v = sp.tile([128, 8], mybir.dt.float32, name="v", tag="v")
    mean, msq_eps, vareps, std, rec, t1, s2_col, t_col = (
        v[:, i:i + 1] for i in range(8))
    nc.vector.tensor_scalar(mean, g[:, 0:1], 2.0 / M_GLOBAL, None, op0=ALU.mult)
    # msq_eps = mean^2 - eps
    nc.vector.tensor_scalar(msq_eps, mean, mean, EPS, op0=ALU.mult, op1=ALU.subtract)
    # vareps = 4*ssq/M - (mean^2 - eps) = var + eps
    nc.vector.scalar_tensor_tensor(
        vareps, g[:, 1:2], 4.0 / M_GLOBAL, msq_eps,
        op0=ALU.mult, op1=ALU.subtract)
    nc.scalar.activation(std, vareps, AF.Sqrt)
    # one Newton step: std = 0.5*(std + vareps/std)
    nc.vector.reciprocal(rec, std)
    # t1 = 0.5 * vareps / std
    nc.vector.tensor_scalar(t1, rec, vareps, 0.5, op0=ALU.mult, op1=ALU.mult)
    nc.vector.scalar_tensor_tensor(std, std, 0.5, t1,
                                   op0=ALU.mult, op1=ALU.add)
    nc.vector.reciprocal(rec, std)
    # s2 = 2 * gamma / std
    nc.vector.tensor_scalar(s2_col, rec, gbt[:, 0:1], 2.0,
                            op0=ALU.mult, op1=ALU.mult)
    # t = beta - s2*mean/2
    nc.vector.tensor_scalar(t1, mean, -0.5, None, op0=ALU.mult)
    nc.vector.scalar_tensor_tensor(
        t_col, s2_col, t1, gbt[:, 1:2], op0=ALU.mult, op1=ALU.add)

    # ---- pass 2: normalize + store. Two batches per output tile (fewer,
    # bigger DMAs); out-DMAs alternate across both HWDGE queues ----
    for bp in range(NB // 2):
        ot = op_pool.tile([128, 2 * LO], mybir.dt.float32, name=f"ot{bp}", tag="ot")
        for sub in range(2):
            nc.scalar.activation(
                ot[:, sub * LO:(sub + 1) * LO], y_tiles[2 * bp + sub][:],
                AF.Identity, bias=t_col, scale=s2_col)
        eng = nc.sync if bp % 2 == 0 else nc.scalar
        eng.dma_start(
            out.rearrange("(a p) l -> p a l", p=128)[:, 2 * bp:2 * bp + 2, :],
            ot.rearrange("p (a l) -> p a l", a=2))


def _prep_weights(W: np.ndarray) -> np.ndarray:
    """Host-side: pack the 8 DoubleRow lhsT matrices [128, 2, 128] fp8:
    (parity even/odd) x (conv O/E) x (j 0/1). Partition k=(c,g): channel
    c=k%64, shift s(k) (0/2 direct/shifted, swapped for odd parity); pair
    element i is tap 4j + i + s(k)."""
    bw = np.sign(W).astype(np.float32)          # [CO, CI, K]
    wh = np.zeros((CO, CI, 8), np.float32)
    wh[:, :, :K] = bw
    wo = np.zeros((CO, CI, 8), np.float32)      # O-conv taps: w[t-1]
    wo[:, :, 1:8] = wh[:, :, 0:7]

    wt = np.zeros((128, 8, 2, 128), np.float32)  # [k, set, i, o]
    ks = np.arange(128)
    cs = ks % 64
    for parity in range(2):
        s_of_k = np.where(ks < 64, 0, 2) if parity == 0 else \
            np.where(ks < 64, 2, 0)
        for conv_i, wsrc in enumerate([wo, wh]):
            for j in range(2):
                si = (parity * 2 + conv_i) * 2 + j
                for i in range(2):
                    t = 4 * j + i + s_of_k          # [128]
                    wt[ks, si, i, :] = wsrc[:, cs, t].T
    return wt.reshape(128, 8 * 256).astype(FP8NP)


_NC_CACHE = {}


def kernel(x, W, prelu_w, gamma, beta):
    x = np.asarray(x)
    W = np.asarray(W)
    alpha = float(np.asarray(prelu_w).reshape(-1)[0])
    gamma = np.asarray(gamma, dtype=np.float32)
    beta = np.asarray(beta, dtype=np.float32)

    assert x.shape == (N, CI, L), x.shape
    wts = _prep_weights(W)
    gb = np.stack([gamma, beta], axis=1).astype(np.float32)

    key = alpha
    if key not in _NC_CACHE:
        _NC_CACHE[key] = _build(alpha)
    nc = _NC_CACHE[key]

    in_maps = []
    for c in range(N_CORES):
        shard = np.ascontiguousarray(
            x[c * NB:(c + 1) * NB].reshape(NB * CI, L), dtype=np.float32)
        in_maps.append({"xs": shard, "wts": wts, "gb": gb})

    res = run_bass_kernel_spmd(nc, in_maps, core_ids=list(range(N_CORES)))
    outs = [res.results[c]["out"].reshape(NB, CO, LO) for c in range(N_CORES)]
    return np.concatenate(outs, axis=0)
